# revision 74
# baseline (speedup 1.0000x reference)
"""Fused linear + cross-entropy loss (BaseChunkLoss) on 8 trn2 NeuronCores.

Strategy: 2-way token x 4-way vocab sharding (grid (i, j), core c = i*4 + j).
  - Tokens (N=8192) split in 2 halves of T=4096; vocab (V=32000) split in 4
    quarters of VC=8000. Each core computes the partial exp-sum of its token
    half over its vocab quarter; the host adds the 4 partials per token and
    takes log (the cross-device logsumexp of the sharding hint, done on the
    scalar-sized partials host-side, standing in for the wrapper's
    all_reduce).
  - Per-core HBM traffic is ~46 MB (fp8 weights quarter + fp8 hidden half +
    f32 rowdot operands + broadcast bias), far below the tensor-engine time,
    so the kernel runs at the PE roofline: fp8e4 DoubleRow matmuls (K=256
    per pass, 0.5 cycles/column) = ~427 us of PE work per core.
  - Quantization to fp8 (weights pre-scaled by 64 for e4m3 range) happens on
    host; the device descales during the bias add, exactly matching the
    numerics of the on-device-converted fp8 baseline (~7.6e-5 rel err).
  - The target logit is computed exactly in f32: host gathers W[labels], the
    device does the per-token rowdot with an accumulating DVE
    scalar_tensor_tensor spread through group 1's slack. Bias added on host.

Device pipeline per (vocab-group g, token-block m, half): 16 DoubleRow
matmuls accumulate psum [128 tok x 2 banks x 500 vocab]; DVE does
(psum/64 + bias) in place; ACT computes exp with a fused row-sum
accumulator into s_cols. Per-bank-pair psum tiles (4 in flight) keep the
drain chain off the PE critical path.
"""
import numpy as np
import ml_dtypes
from contextlib import ExitStack

from concourse import bacc, mybir, tile
from concourse.bass_utils import run_bass_kernel_spmd

F32 = mybir.dt.float32
FP8 = mybir.dt.float8e4
Alu = mybir.AluOpType
Act = mybir.ActivationFunctionType

N_CORES = 8
N_TOK = 8192
D = 2048
V = 32000
P = 128

TOK_SPLIT = 2
VOC_SPLIT = 4
T = N_TOK // TOK_SPLIT        # 4096 tokens per core
VC = V // VOC_SPLIT           # 8000 vocab per core
KP = D // 256                 # 8 DoubleRow contraction passes of K=256
GV = 2000                     # vocab columns per W group (4 psum banks)
NG = VC // GV                 # 4 groups per core
MB = T // P                   # 32 token blocks per core
MBQ = MB // VOC_SPLIT         # 8 rowdot token blocks per core (1024 tokens)
BANK = 500                    # columns per psum bank

W_SCALE = 64.0                # fp8 weight pre-scale (e4m3 range)
FP8NP = ml_dtypes.float8_e4m3


def _build():
    nc = bacc.Bacc("TRN2", target_bir_lowering=False, debug=False)
    h_d = nc.declare_dram_parameter("h", [P, KP, 2, T], FP8, isOutput=False)
    h0a_d = nc.declare_dram_parameter("h0a", [P, KP * 2 * 256], FP8, isOutput=False)
    h0b_d = nc.declare_dram_parameter("h0b", [P, KP * 2 * 256], FP8, isOutput=False)
    h0_d = nc.declare_dram_parameter("h0", [P, KP * 2 * 256], FP8, isOutput=False)
    W_d = nc.declare_dram_parameter("W", [P, KP, 2, VC], FP8, isOutput=False)
    bias_d = nc.declare_dram_parameter("bias", [VC], F32, isOutput=False)
    brow_d = nc.declare_dram_parameter("brow", [GV // 2], mybir.dt.bfloat16,
                                       isOutput=False)
    brow2_d = nc.declare_dram_parameter("brow2", [GV // 2], mybir.dt.bfloat16,
                                        isOutput=False)
    hn_d = nc.declare_dram_parameter("hn", [MBQ * P, D], F32, isOutput=False)
    wg_d = nc.declare_dram_parameter("wg", [MBQ * P, D], F32, isOutput=False)
    s_out = nc.declare_dram_parameter("s_out", [P, MB], F32, isOutput=True)
    s_out2 = nc.declare_dram_parameter("s_out2", [P, NG * 2 + 1], F32,
                                       isOutput=True)
    t_out = nc.declare_dram_parameter("t_out", [P, MBQ], F32, isOutput=True)

    # h streams in token-chunks: each chunk carries ALL contraction passes
    # for 4 m-blocks, so the pipeline reaches full rate after ~1 MB of h
    HC = 512                                           # tokens per h chunk
    h_r = h_d[:].rearrange("p kp j (c t) -> c p kp j t", t=HC)
    W_r = W_d[:]                                       # [128, KP, 2, VC]

    with tile.TileContext(nc) as tc, ExitStack() as ctx:
        hpool = ctx.enter_context(tc.tile_pool(name="hT", bufs=1))
        wpool = ctx.enter_context(tc.tile_pool(name="w", bufs=2))
        bpool = ctx.enter_context(tc.tile_pool(name="bias", bufs=2))
        pspool = ctx.enter_context(tc.tile_pool(name="ps", bufs=4, space="PSUM"))
        epool = ctx.enter_context(tc.tile_pool(name="ejunk", bufs=2))
        hgpool = ctx.enter_context(tc.tile_pool(name="hg", bufs=2))
        wgpool = ctx.enter_context(tc.tile_pool(name="wgt", bufs=2))
        djpool = ctx.enter_context(tc.tile_pool(name="dj", bufs=1))
        acc = ctx.enter_context(tc.tile_pool(name="acc", bufs=1))

        s_cols = acc.tile([P, MB * NG * 2 + 1], F32, tag="scols")
        s_fin = acc.tile([P, MB], F32, tag="sfin")
        t_fin = acc.tile([P, MBQ], F32, tag="tfin")

        # Warm-up: dummy f32 matmuls on a zeroed tile keep PE busy (and ramp
        # its p-state to max) while the first real operands stream in. They
        # finish right as the first W group lands, so real matmuls start at
        # full clock with no idle gap.
        warm_state = {}

        def emit_warm(n_big, n_small):
            # dummy matmuls on a zeroed tile keep PE busy and its p-state
            # ramped while real operands stream in; also injected into the
            # first tiles' stutter windows
            if "ws" not in warm_state:
                ws = acc.tile([P, 512], F32, tag="warm")
                nc.vector.memset(ws[:], 0.0)
                wpt = pspool.tile([P, 2, 512], F32, tag="ps", name="wpt")
                warm_state["ws"] = ws
                warm_state["pt"] = wpt
            ws, wpt = warm_state["ws"], warm_state["pt"]
            for i in range(n_big):
                nc.tensor.matmul(wpt[:, i % 2, 0:512], ws[:, 0:128],
                                 ws[:, 0:512], start=True, stop=True)
            for i in range(n_small):
                nc.tensor.matmul(wpt[:, i % 2, 0:128], ws[:, 0:128],
                                 ws[:, 0:128], start=True, stop=True)

        # bias cols 0:1000 as one 4KB row -> on-chip broadcast via a K=1
        # ones-vector matmul into psum (PE is idle during startup anyway);
        # this drops the 0.5MB broadcast DMA from the gating transfer chain

        # preload the Exp activation table off the critical path: the first
        # real activation would otherwise pay a ~1.3us LoadActFuncSet inside
        # the first psum drain chain
        az = acc.tile([P, 1], F32, tag="actwarm")
        nc.vector.memset(az[:], 0.0)
        aj = acc.tile([P, 1], F32, tag="actjunk")
        nc.scalar.activation(aj[:], az[:], Act.Exp)

        emit_warm(2, 9)

        # bias cols 0:1000 as one 2KB bf16 row -> on-chip broadcast via a
        # K=1 ones-vector matmul (PE is idle during startup anyway); this
        # drops the 0.5MB broadcast DMA from the gating transfer chain
        ones = acc.tile([1, P], mybir.dt.bfloat16, tag="ones")
        nc.vector.memset(ones[0:1, :], 1.0)
        brow = acc.tile([1, GV // 2], mybir.dt.bfloat16, tag="brow")
        nc.sync.dma_start(brow[0:1, :], brow_d[:].partition_broadcast(1))
        bpt = pspool.tile([P, 2, 512], F32, tag="ps", name="bpt")
        for bk in range(2):
            nc.tensor.matmul(bpt[:, bk, 0:BANK], ones[0:1, :],
                             brow[0:1, bk * BANK:(bk + 1) * BANK],
                             start=True, stop=True)

        # startup order tuned for earliest full-rate PE: first token chunk,
        # then W group 0 in 512-col slivers (512 keeps full DMA bandwidth),
        # bias afterwards (the psum ring gives the first drains slack)
        # group 0 runs all half-0 tiles first (cols 0:1000), so only the
        # first two W slivers + first bias half gate the pipeline; the rest
        # streams in far ahead of its consumption
        hT = hpool.tile([P, KP, 2, T], FP8, tag="hT")
        h0T = acc.tile([P, KP, 2, 256], FP8, tag="h0T")
        wv0 = wpool.tile([P, KP, 2, GV], FP8, tag="w")
        bb0 = bpool.tile([P, GV], F32, tag="bias")
        nc.sync.dma_start(wv0[:, :, :, 0:512], W_r[:, :, :, 0:512])
        nc.sync.dma_start(
            h0T[:], h0_d[:].rearrange("p (kp j t) -> p kp j t", kp=KP, j=2))
        nc.sync.dma_start(wv0[:, :, :, 512:1024], W_r[:, :, :, 512:1024])
        nc.sync.dma_start(hT[:, :, :, 0:HC], h_r[0])
        nc.sync.dma_start(bb0[:, 0:GV // 2], bias_d[0:GV // 2].partition_broadcast(P))
        for c in range(1, T // HC):
            nc.sync.dma_start(hT[:, :, :, c * HC:(c + 1) * HC], h_r[c])
        # cols 1000:2000 and the second bias half are only consumed by the
        # half-1 sweep of group 0 (~60us in) -- load them after the h chunks
        nc.sync.dma_start(wv0[:, :, :, 1024:GV], W_r[:, :, :, 1024:GV])
        nc.sync.dma_start(bb0[:, GV // 2:GV],
                          bias_d[GV // 2:GV].partition_broadcast(P))

        # prefetch group 1 and the rowdot operands behind it
        wv1 = wpool.tile([P, KP, 2, GV], FP8, tag="w")
        nc.sync.dma_start(wv1[:], W_r[:, :, :, GV:2 * GV])
        bb1 = bpool.tile([P, GV], F32, tag="bias")
        nc.sync.dma_start(bb1[:], bias_d[GV:2 * GV].partition_broadcast(P))
        rowdot_io = []
        for mb in range(MBQ):
            hg = hgpool.tile([P, D], F32, tag="hg")
            nc.sync.dma_start(hg[:], hn_d[mb * P:(mb + 1) * P, :])
            wgt = wgpool.tile([P, D], F32, tag="wgt")
            nc.sync.dma_start(wgt[:], wg_d[mb * P:(mb + 1) * P, :])
            rowdot_io.append((hg, wgt))

        wtiles = [wv0, wv1]
        btiles = [bb0, bb1]
        for g in range(NG):
            wv, bb = wtiles[g], btiles[g]
            if g + 2 < NG:          # keep the double-buffer one group ahead
                wnx = wpool.tile([P, KP, 2, GV], FP8, tag="w")
                nc.sync.dma_start(
                    wnx[:], W_r[:, :, :, (g + 2) * GV:(g + 3) * GV])
                bnx = bpool.tile([P, GV], F32, tag="bias")
                nc.sync.dma_start(
                    bnx[:], bias_d[(g + 2) * GV:(g + 3) * GV].partition_broadcast(P))
                wtiles.append(wnx)
                btiles.append(bnx)

            def emit_rowdot(mb):
                # exact-f32 target logits: t = sum_d hn * W[label], one fused
                # DVE op per token block, spread through group 1's m-loop to
                # fit the per-tile DVE slack
                hg, wgt = rowdot_io[mb]
                dj = djpool.tile([P, D], F32, tag="dj", name="dj")
                nc.vector.scalar_tensor_tensor(
                    dj[:], hg[:], 1.0, wgt[:],
                    op0=Alu.mult, op1=Alu.mult,
                    accum_out=t_fin[:, mb:mb + 1])

            def emit_tile(m, half, split=False):
                if g == 0 and half == 0 and m < 2:
                    # first two blocks read the small contiguous h0 landing
                    lhsT = h0T[:, :, :, m * P:(m + 1) * P]
                else:
                    lhsT = hT[:, :, :, m * P:(m + 1) * P]
                col = m * (NG * 2) + g * 2 + half
                bbv = bb[:, half * 2 * BANK:(half + 1) * 2 * BANK]
                bbv = bbv.rearrange("p (b c) -> p b c", c=BANK)
                for bk in range(2):
                    if not split and bk == 1:
                        break
                    pt = pspool.tile([P, 2, 512], F32, tag="ps", name="pt")
                    bks = [bk] if split else [0, 1]
                    for b in bks:
                        c0 = half * (2 * BANK) + b * BANK
                        for kp in range(KP):
                            nc.tensor.matmul(
                                pt[:, b, 0:BANK], lhsT[:, kp, :, :],
                                wv[:, kp, :, c0:c0 + BANK],
                                start=(kp == 0), stop=(kp == KP - 1),
                                perf_mode=mybir.MatmulPerfMode.DoubleRow,
                            )
                    if split:
                        # 1-bank tiles: bank 0 drains while bank 1 still
                        # accumulates, halving the end-of-program chain
                        psb = pt[:, bk:bk + 1, 0:BANK]
                        nc.vector.scalar_tensor_tensor(
                            psb, psb, 1.0 / W_SCALE, bbv[:, bk:bk + 1, :],
                            op0=Alu.mult, op1=Alu.add)
                        et = epool.tile([P, 2, BANK], F32, tag="ejunk",
                                        name="et")
                        cb = col + bk
                        nc.scalar.activation(
                            et[:, 0:1, :], psb, Act.Exp,
                            accum_out=s_cols[:, cb:cb + 1])
                    else:
                        psl = pt[:, 0:2, 0:BANK]
                        nc.vector.scalar_tensor_tensor(
                            psl, psl, 1.0 / W_SCALE, bbv,
                            op0=Alu.mult, op1=Alu.add)
                        et = epool.tile([P, 2, BANK], F32, tag="ejunk",
                                        name="et")
                        nc.scalar.activation(
                            et[:], psl, Act.Exp,
                            accum_out=s_cols[:, col:col + 1])

            if g == 0:
                # m0-3 h0 first (gated only by h0a/h0b + W cols 0:1024),
                # then m0-3 h1 (gated by the early s3), then the sweeps
                for m in range(4):
                    emit_tile(m, 0)
                bpt2 = pspool.tile([P, 2, 512], F32, tag="ps", name="bpt2")
                for bk in range(2):
                    nc.tensor.matmul(bpt2[:, bk, 0:BANK], ones[0:1, :],
                                     brow2[0:1, bk * BANK:(bk + 1) * BANK],
                                     start=True, stop=True)
                nc.vector.tensor_copy(
                    bb0[:, GV // 2:GV].rearrange("p (b c) -> p b c", c=BANK),
                    bpt2[:, 0:2, 0:BANK])
                for m in range(4):
                    emit_tile(m, 1)
                for half in range(2):
                    for m in range(4, MB):
                        emit_tile(m, half)
            else:
                if g == 2:
                    # t_fin is complete after group 1's rowdots; get its
                    # writeback off the critical-path tail
                    nc.sync.dma_start(t_out[:], t_fin[:])
                for m in range(MB):
                    if g == 1 and m % 4 == 0:
                        emit_rowdot(m // 4)
                    for half in range(2):
                        last = (g == NG - 1 and m == MB - 1 and half == 1)
                        emit_tile(m, half, split=last)
                    if g == NG - 1 and m < MB - 1:
                        nc.vector.tensor_reduce(
                            s_fin[:, m:m + 1],
                            s_cols[:, m * (NG * 2):(m + 1) * (NG * 2)],
                            axis=mybir.AxisListType.X, op=Alu.add)
                    if g == NG - 1 and m == MB - 2:
                        # all but the last token block are final: write them
                        # back now so only the last block's raw partials
                        # remain on the end-of-program critical path
                        nc.sync.dma_start(s_out[:, 0:MB - 1], s_fin[:, 0:MB - 1])

        # last token block: ship the 9 raw partials (the split final tile
        # uses two columns); host reduces
        nc.scalar.dma_start(
            s_out2[:], s_cols[:, (MB - 1) * (NG * 2):MB * (NG * 2) + 1])

    nc.compile()
    return nc


_NC_CACHE = {}


def _get_program():
    if "nc" not in _NC_CACHE:
        _NC_CACHE["nc"] = _build()
    return _NC_CACHE["nc"]


def _to_sbuf_layout(a):
    """[D, X] f32/fp8 -> [128, KP, 2, X] matching d = kp*256 + j*128 + ki."""
    X = a.shape[1]
    return np.ascontiguousarray(
        a.reshape(KP, 2, P, X).transpose(2, 0, 1, 3))


def kernel(hidden_states, head_weight, head_bias, loss_weight, labels,
           chunk_size=None, **_unused):
    hidden = np.asarray(hidden_states, dtype=np.float32)
    W = np.asarray(head_weight, dtype=np.float32)
    bias = np.asarray(head_bias, dtype=np.float32)
    lw = np.asarray(loss_weight, dtype=np.float32)
    labels = np.asarray(labels).astype(np.int64)

    assert hidden.shape == (N_TOK, D) and W.shape == (V, D)

    nc = _get_program()

    hq = hidden.astype(FP8NP)                       # [N, D] fp8
    Wq = (W * W_SCALE).astype(FP8NP)                # [V, D] fp8, x64
    Wg = W[labels]                                  # gathered rows [N, D] f32

    in_maps = []
    for c in range(N_CORES):
        i, j = divmod(c, VOC_SPLIT)
        tok = slice(i * T, (i + 1) * T)
        voc = slice(j * VC, (j + 1) * VC)
        # rowdot tokens: quarter j of token half i
        rtok = slice(i * T + j * MBQ * P, i * T + (j + 1) * MBQ * P)
        hsb = _to_sbuf_layout(hq[tok].T)
        in_maps.append(dict(
            h=hsb,
            h0=np.ascontiguousarray(hsb[:, :, :, 0:256]).reshape(P, -1),
            W=_to_sbuf_layout(Wq[voc].T),
            bias=np.ascontiguousarray(bias[voc]),
            brow=np.ascontiguousarray(bias[voc][0:GV // 2]).astype(ml_dtypes.bfloat16),
            brow2=np.ascontiguousarray(bias[voc][GV // 2:GV]).astype(ml_dtypes.bfloat16),
            hn=np.ascontiguousarray(hidden[rtok]),
            wg=np.ascontiguousarray(Wg[rtok]),
        ))
    res = run_bass_kernel_spmd(nc, in_maps, list(range(N_CORES)))

    # unshard + host-side combine (the scalar all_reduce of the hint):
    # sum the 4 vocab-quarter exp-sums per token, then logsumexp
    s = np.zeros((TOK_SPLIT, T), np.float64)
    tgt = np.zeros(N_TOK, np.float64)
    for c in range(N_CORES):
        i, j = divmod(c, VOC_SPLIT)
        r = res.results[c]
        sc = r["s_out"].astype(np.float64)          # [P, MB], token = m*128+p
        sc[:, MB - 1] = r["s_out2"].astype(np.float64).sum(-1)
        s[i] += sc.T.reshape(-1)
        rtok = slice(i * T + j * MBQ * P, i * T + (j + 1) * MBQ * P)
        tgt[rtok] = r["t_out"].T.reshape(-1).astype(np.float64)
    lse = np.log(s.reshape(-1))
    tgt = tgt + bias[labels].astype(np.float64)     # rowdot excludes bias
    nll = lse - tgt
    w64 = lw.astype(np.float64)
    loss = (w64 * nll).sum() / max(w64.sum(), 1.0)
    return np.float32(loss)



# revision 75
# speedup vs baseline: 1.0037x; 1.0037x over previous
"""Fused linear + cross-entropy loss (BaseChunkLoss) on 8 trn2 NeuronCores.

Strategy: 2-way token x 4-way vocab sharding (grid (i, j), core c = i*4 + j).
  - Tokens (N=8192) split in 2 halves of T=4096; vocab (V=32000) split in 4
    quarters of VC=8000. Each core computes the partial exp-sum of its token
    half over its vocab quarter; the host adds the 4 partials per token and
    takes log (the cross-device logsumexp of the sharding hint, done on the
    scalar-sized partials host-side, standing in for the wrapper's
    all_reduce).
  - Per-core HBM traffic is ~46 MB (fp8 weights quarter + fp8 hidden half +
    f32 rowdot operands + broadcast bias), far below the tensor-engine time,
    so the kernel runs at the PE roofline: fp8e4 DoubleRow matmuls (K=256
    per pass, 0.5 cycles/column) = ~427 us of PE work per core.
  - Quantization to fp8 (weights pre-scaled by 64 for e4m3 range) happens on
    host; the device descales during the bias add, exactly matching the
    numerics of the on-device-converted fp8 baseline (~7.6e-5 rel err).
  - The target logit is computed exactly in f32: host gathers W[labels], the
    device does the per-token rowdot with an accumulating DVE
    scalar_tensor_tensor spread through group 1's slack. Bias added on host.

Device pipeline per (vocab-group g, token-block m, half): 16 DoubleRow
matmuls accumulate psum [128 tok x 2 banks x 500 vocab]; DVE does
(psum/64 + bias) in place; ACT computes exp with a fused row-sum
accumulator into s_cols. Per-bank-pair psum tiles (4 in flight) keep the
drain chain off the PE critical path.
"""
import numpy as np
import ml_dtypes
from contextlib import ExitStack

from concourse import bacc, mybir, tile
from concourse.bass_utils import run_bass_kernel_spmd

F32 = mybir.dt.float32
FP8 = mybir.dt.float8e4
Alu = mybir.AluOpType
Act = mybir.ActivationFunctionType

N_CORES = 8
N_TOK = 8192
D = 2048
V = 32000
P = 128

TOK_SPLIT = 2
VOC_SPLIT = 4
T = N_TOK // TOK_SPLIT        # 4096 tokens per core
VC = V // VOC_SPLIT           # 8000 vocab per core
KP = D // 256                 # 8 DoubleRow contraction passes of K=256
GV = 2000                     # vocab columns per W group (4 psum banks)
NG = VC // GV                 # 4 groups per core
MB = T // P                   # 32 token blocks per core
MBQ = MB // VOC_SPLIT         # 8 rowdot token blocks per core (1024 tokens)
BANK = 500                    # columns per psum bank

W_SCALE = 64.0                # fp8 weight pre-scale (e4m3 range)
FP8NP = ml_dtypes.float8_e4m3


def _build():
    nc = bacc.Bacc("TRN2", target_bir_lowering=False, debug=False)
    h_d = nc.declare_dram_parameter("h", [P, KP, 2, T], FP8, isOutput=False)
    h0a_d = nc.declare_dram_parameter("h0a", [P, KP * 2 * 256], FP8, isOutput=False)
    h0b_d = nc.declare_dram_parameter("h0b", [P, KP * 2 * 256], FP8, isOutput=False)
    h0_d = nc.declare_dram_parameter("h0", [P, KP * 2 * 256], FP8, isOutput=False)
    W_d = nc.declare_dram_parameter("W", [P, KP, 2, VC], FP8, isOutput=False)
    bias_d = nc.declare_dram_parameter("bias", [VC], F32, isOutput=False)
    brow_d = nc.declare_dram_parameter("brow", [GV // 2], mybir.dt.bfloat16,
                                       isOutput=False)
    brow2_d = nc.declare_dram_parameter("brow2", [GV // 2], mybir.dt.bfloat16,
                                        isOutput=False)
    hn_d = nc.declare_dram_parameter("hn", [MBQ * P, D], F32, isOutput=False)
    wg_d = nc.declare_dram_parameter("wg", [MBQ * P, D], F32, isOutput=False)
    s_out = nc.declare_dram_parameter("s_out", [P, MB], F32, isOutput=True)
    s_out2 = nc.declare_dram_parameter("s_out2", [P, NG * 2 + 1], F32,
                                       isOutput=True)
    t_out = nc.declare_dram_parameter("t_out", [P, MBQ], F32, isOutput=True)

    # h streams in token-chunks: each chunk carries ALL contraction passes
    # for 4 m-blocks, so the pipeline reaches full rate after ~1 MB of h
    HC = 512                                           # tokens per h chunk
    h_r = h_d[:].rearrange("p kp j (c t) -> c p kp j t", t=HC)
    W_r = W_d[:]                                       # [128, KP, 2, VC]

    with tile.TileContext(nc) as tc, ExitStack() as ctx:
        hpool = ctx.enter_context(tc.tile_pool(name="hT", bufs=1))
        wpool = ctx.enter_context(tc.tile_pool(name="w", bufs=2))
        bpool = ctx.enter_context(tc.tile_pool(name="bias", bufs=2))
        pspool = ctx.enter_context(tc.tile_pool(name="ps", bufs=4, space="PSUM"))
        epool = ctx.enter_context(tc.tile_pool(name="ejunk", bufs=2))
        hgpool = ctx.enter_context(tc.tile_pool(name="hg", bufs=2))
        wgpool = ctx.enter_context(tc.tile_pool(name="wgt", bufs=2))
        djpool = ctx.enter_context(tc.tile_pool(name="dj", bufs=1))
        acc = ctx.enter_context(tc.tile_pool(name="acc", bufs=1))

        s_cols = acc.tile([P, MB * NG * 2 + 1], F32, tag="scols")
        s_fin = acc.tile([P, MB], F32, tag="sfin")
        t_fin = acc.tile([P, MBQ], F32, tag="tfin")

        # Warm-up: dummy f32 matmuls on a zeroed tile keep PE busy (and ramp
        # its p-state to max) while the first real operands stream in. They
        # finish right as the first W group lands, so real matmuls start at
        # full clock with no idle gap.
        warm_state = {}

        def emit_warm(n_big, n_small):
            # dummy matmuls on a zeroed tile keep PE busy and its p-state
            # ramped while real operands stream in; also injected into the
            # first tiles' stutter windows
            if "ws" not in warm_state:
                ws = acc.tile([P, 512], F32, tag="warm")
                nc.vector.memset(ws[:], 0.0)
                wpt = pspool.tile([P, 2, 512], F32, tag="ps", name="wpt")
                warm_state["ws"] = ws
                warm_state["pt"] = wpt
            ws, wpt = warm_state["ws"], warm_state["pt"]
            for i in range(n_big):
                nc.tensor.matmul(wpt[:, i % 2, 0:512], ws[:, 0:128],
                                 ws[:, 0:512], start=True, stop=True)
            for i in range(n_small):
                nc.tensor.matmul(wpt[:, i % 2, 0:128], ws[:, 0:128],
                                 ws[:, 0:128], start=True, stop=True)

        # bias cols 0:1000 as one 4KB row -> on-chip broadcast via a K=1
        # ones-vector matmul into psum (PE is idle during startup anyway);
        # this drops the 0.5MB broadcast DMA from the gating transfer chain

        # preload the Exp activation table off the critical path: the first
        # real activation would otherwise pay a ~1.3us LoadActFuncSet inside
        # the first psum drain chain
        az = acc.tile([P, 1], F32, tag="actwarm")
        nc.vector.memset(az[:], 0.0)
        aj = acc.tile([P, 1], F32, tag="actjunk")
        nc.scalar.activation(aj[:], az[:], Act.Exp)

        emit_warm(2, 9)

        # bias cols 0:1000 as one 2KB bf16 row -> on-chip broadcast via a
        # K=1 ones-vector matmul (PE is idle during startup anyway); this
        # drops the 0.5MB broadcast DMA from the gating transfer chain
        ones = acc.tile([1, P], mybir.dt.bfloat16, tag="ones")
        nc.vector.memset(ones[0:1, :], 1.0)
        brow = acc.tile([1, GV // 2], mybir.dt.bfloat16, tag="brow")
        nc.sync.dma_start(brow[0:1, :], brow_d[:].partition_broadcast(1))
        bpt = pspool.tile([P, 2, 512], F32, tag="ps", name="bpt")
        for bk in range(2):
            nc.tensor.matmul(bpt[:, bk, 0:BANK], ones[0:1, :],
                             brow[0:1, bk * BANK:(bk + 1) * BANK],
                             start=True, stop=True)

        # startup order tuned for earliest full-rate PE: first token chunk,
        # then W group 0 in 512-col slivers (512 keeps full DMA bandwidth),
        # bias afterwards (the psum ring gives the first drains slack)
        # group 0 runs all half-0 tiles first (cols 0:1000), so only the
        # first two W slivers + first bias half gate the pipeline; the rest
        # streams in far ahead of its consumption
        hT = hpool.tile([P, KP, 2, T], FP8, tag="hT")
        h0T = acc.tile([P, KP, 2, 256], FP8, tag="h0T")
        wv0 = wpool.tile([P, KP, 2, GV], FP8, tag="w")
        bb0 = bpool.tile([P, GV], F32, tag="bias")
        nc.sync.dma_start(wv0[:, :, :, 0:512], W_r[:, :, :, 0:512])
        nc.sync.dma_start(
            h0T[:], h0_d[:].rearrange("p (kp j t) -> p kp j t", kp=KP, j=2))
        nc.sync.dma_start(wv0[:, :, :, 512:1024], W_r[:, :, :, 512:1024])
        nc.sync.dma_start(hT[:, :, :, 0:HC], h_r[0])
        nc.sync.dma_start(bb0[:, 0:GV // 2], bias_d[0:GV // 2].partition_broadcast(P))
        for c in range(1, T // HC):
            nc.sync.dma_start(hT[:, :, :, c * HC:(c + 1) * HC], h_r[c])
        # cols 1000:2000 and the second bias half are only consumed by the
        # half-1 sweep of group 0 (~60us in) -- load them after the h chunks
        nc.sync.dma_start(wv0[:, :, :, 1024:GV], W_r[:, :, :, 1024:GV])
        nc.sync.dma_start(bb0[:, GV // 2:GV],
                          bias_d[GV // 2:GV].partition_broadcast(P))

        # prefetch group 1 and the rowdot operands behind it
        wv1 = wpool.tile([P, KP, 2, GV], FP8, tag="w")
        nc.sync.dma_start(wv1[:], W_r[:, :, :, GV:2 * GV])
        bb1 = bpool.tile([P, GV], F32, tag="bias")
        nc.sync.dma_start(bb1[:], bias_d[GV:2 * GV].partition_broadcast(P))
        rowdot_io = []
        for mb in range(MBQ):
            hg = hgpool.tile([P, D], F32, tag="hg")
            nc.sync.dma_start(hg[:], hn_d[mb * P:(mb + 1) * P, :])
            wgt = wgpool.tile([P, D], F32, tag="wgt")
            nc.sync.dma_start(wgt[:], wg_d[mb * P:(mb + 1) * P, :])
            rowdot_io.append((hg, wgt))

        wtiles = [wv0, wv1]
        btiles = [bb0, bb1]
        for g in range(NG):
            wv, bb = wtiles[g], btiles[g]
            if g + 2 < NG:          # keep the double-buffer one group ahead
                wnx = wpool.tile([P, KP, 2, GV], FP8, tag="w")
                nc.sync.dma_start(
                    wnx[:], W_r[:, :, :, (g + 2) * GV:(g + 3) * GV])
                bnx = bpool.tile([P, GV], F32, tag="bias")
                nc.sync.dma_start(
                    bnx[:], bias_d[(g + 2) * GV:(g + 3) * GV].partition_broadcast(P))
                wtiles.append(wnx)
                btiles.append(bnx)

            def emit_rowdot(mb):
                # exact-f32 target logits: t = sum_d hn * W[label], one fused
                # DVE op per token block, spread through group 1's m-loop to
                # fit the per-tile DVE slack
                hg, wgt = rowdot_io[mb]
                dj = djpool.tile([P, D], F32, tag="dj", name="dj")
                nc.vector.scalar_tensor_tensor(
                    dj[:], hg[:], 1.0, wgt[:],
                    op0=Alu.mult, op1=Alu.mult,
                    accum_out=t_fin[:, mb:mb + 1])

            def emit_tile(m, half, split=False):
                if g == 0 and half == 0 and m < 2:
                    # first two blocks read the small contiguous h0 landing
                    lhsT = h0T[:, :, :, m * P:(m + 1) * P]
                else:
                    lhsT = hT[:, :, :, m * P:(m + 1) * P]
                col = m * (NG * 2) + g * 2 + half
                bbv = bb[:, half * 2 * BANK:(half + 1) * 2 * BANK]
                bbv = bbv.rearrange("p (b c) -> p b c", c=BANK)
                for bk in range(2):
                    if not split and bk == 1:
                        break
                    pt = pspool.tile([P, 2, 512], F32, tag="ps", name="pt")
                    bks = [bk] if split else [0, 1]
                    for b in bks:
                        c0 = half * (2 * BANK) + b * BANK
                        for kp in range(KP):
                            nc.tensor.matmul(
                                pt[:, b, 0:BANK], lhsT[:, kp, :, :],
                                wv[:, kp, :, c0:c0 + BANK],
                                start=(kp == 0), stop=(kp == KP - 1),
                                perf_mode=mybir.MatmulPerfMode.DoubleRow,
                            )
                    if split:
                        # 1-bank tiles: bank 0 drains while bank 1 still
                        # accumulates, halving the end-of-program chain
                        psb = pt[:, bk:bk + 1, 0:BANK]
                        nc.vector.scalar_tensor_tensor(
                            psb, psb, 1.0 / W_SCALE, bbv[:, bk:bk + 1, :],
                            op0=Alu.mult, op1=Alu.add)
                        et = epool.tile([P, 2, BANK], F32, tag="ejunk",
                                        name="et")
                        cb = col + bk
                        nc.scalar.activation(
                            et[:, 0:1, :], psb, Act.Exp,
                            accum_out=s_cols[:, cb:cb + 1])
                    else:
                        psl = pt[:, 0:2, 0:BANK]
                        nc.vector.scalar_tensor_tensor(
                            psl, psl, 1.0 / W_SCALE, bbv,
                            op0=Alu.mult, op1=Alu.add)
                        et = epool.tile([P, 2, BANK], F32, tag="ejunk",
                                        name="et")
                        nc.scalar.activation(
                            et[:], psl, Act.Exp,
                            accum_out=s_cols[:, col:col + 1])

            if g == 0:
                # m0-3 h0 first (gated only by h0a/h0b + W cols 0:1024),
                # then m0-3 h1 (gated by the early s3), then the sweeps
                for m in range(8):
                    emit_tile(m, 0)
                bpt2 = pspool.tile([P, 2, 512], F32, tag="ps", name="bpt2")
                for bk in range(2):
                    nc.tensor.matmul(bpt2[:, bk, 0:BANK], ones[0:1, :],
                                     brow2[0:1, bk * BANK:(bk + 1) * BANK],
                                     start=True, stop=True)
                nc.vector.tensor_copy(
                    bb0[:, GV // 2:GV].rearrange("p (b c) -> p b c", c=BANK),
                    bpt2[:, 0:2, 0:BANK])
                for m in range(4):
                    emit_tile(m, 1)
                for m in range(8, MB):
                    emit_tile(m, 0)
                for m in list(range(4, 8)) + list(range(8, MB)):
                    emit_tile(m, 1)
            else:
                if g == 2:
                    # t_fin is complete after group 1's rowdots; get its
                    # writeback off the critical-path tail
                    nc.sync.dma_start(t_out[:], t_fin[:])
                for m in range(MB):
                    if g == 1 and m % 4 == 0:
                        emit_rowdot(m // 4)
                    for half in range(2):
                        last = (g == NG - 1 and m == MB - 1 and half == 1)
                        emit_tile(m, half, split=last)
                    if g == NG - 1 and m < MB - 1:
                        nc.vector.tensor_reduce(
                            s_fin[:, m:m + 1],
                            s_cols[:, m * (NG * 2):(m + 1) * (NG * 2)],
                            axis=mybir.AxisListType.X, op=Alu.add)
                    if g == NG - 1 and m == MB - 2:
                        # all but the last token block are final: write them
                        # back now so only the last block's raw partials
                        # remain on the end-of-program critical path
                        nc.sync.dma_start(s_out[:, 0:MB - 1], s_fin[:, 0:MB - 1])

        # last token block: ship the 9 raw partials (the split final tile
        # uses two columns); host reduces
        nc.scalar.dma_start(
            s_out2[:], s_cols[:, (MB - 1) * (NG * 2):MB * (NG * 2) + 1])

    nc.compile()
    return nc


_NC_CACHE = {}


def _get_program():
    if "nc" not in _NC_CACHE:
        _NC_CACHE["nc"] = _build()
    return _NC_CACHE["nc"]


def _to_sbuf_layout(a):
    """[D, X] f32/fp8 -> [128, KP, 2, X] matching d = kp*256 + j*128 + ki."""
    X = a.shape[1]
    return np.ascontiguousarray(
        a.reshape(KP, 2, P, X).transpose(2, 0, 1, 3))


def kernel(hidden_states, head_weight, head_bias, loss_weight, labels,
           chunk_size=None, **_unused):
    hidden = np.asarray(hidden_states, dtype=np.float32)
    W = np.asarray(head_weight, dtype=np.float32)
    bias = np.asarray(head_bias, dtype=np.float32)
    lw = np.asarray(loss_weight, dtype=np.float32)
    labels = np.asarray(labels).astype(np.int64)

    assert hidden.shape == (N_TOK, D) and W.shape == (V, D)

    nc = _get_program()

    hq = hidden.astype(FP8NP)                       # [N, D] fp8
    Wq = (W * W_SCALE).astype(FP8NP)                # [V, D] fp8, x64
    Wg = W[labels]                                  # gathered rows [N, D] f32

    in_maps = []
    for c in range(N_CORES):
        i, j = divmod(c, VOC_SPLIT)
        tok = slice(i * T, (i + 1) * T)
        voc = slice(j * VC, (j + 1) * VC)
        # rowdot tokens: quarter j of token half i
        rtok = slice(i * T + j * MBQ * P, i * T + (j + 1) * MBQ * P)
        hsb = _to_sbuf_layout(hq[tok].T)
        in_maps.append(dict(
            h=hsb,
            h0=np.ascontiguousarray(hsb[:, :, :, 0:256]).reshape(P, -1),
            W=_to_sbuf_layout(Wq[voc].T),
            bias=np.ascontiguousarray(bias[voc]),
            brow=np.ascontiguousarray(bias[voc][0:GV // 2]).astype(ml_dtypes.bfloat16),
            brow2=np.ascontiguousarray(bias[voc][GV // 2:GV]).astype(ml_dtypes.bfloat16),
            hn=np.ascontiguousarray(hidden[rtok]),
            wg=np.ascontiguousarray(Wg[rtok]),
        ))
    res = run_bass_kernel_spmd(nc, in_maps, list(range(N_CORES)))

    # unshard + host-side combine (the scalar all_reduce of the hint):
    # sum the 4 vocab-quarter exp-sums per token, then logsumexp
    s = np.zeros((TOK_SPLIT, T), np.float64)
    tgt = np.zeros(N_TOK, np.float64)
    for c in range(N_CORES):
        i, j = divmod(c, VOC_SPLIT)
        r = res.results[c]
        sc = r["s_out"].astype(np.float64)          # [P, MB], token = m*128+p
        sc[:, MB - 1] = r["s_out2"].astype(np.float64).sum(-1)
        s[i] += sc.T.reshape(-1)
        rtok = slice(i * T + j * MBQ * P, i * T + (j + 1) * MBQ * P)
        tgt[rtok] = r["t_out"].T.reshape(-1).astype(np.float64)
    lse = np.log(s.reshape(-1))
    tgt = tgt + bias[labels].astype(np.float64)     # rowdot excludes bias
    nll = lse - tgt
    w64 = lw.astype(np.float64)
    loss = (w64 * nll).sum() / max(w64.sum(), 1.0)
    return np.float32(loss)



# revision 76
# speedup vs baseline: 1.0047x; 1.0009x over previous
"""Fused linear + cross-entropy loss (BaseChunkLoss) on 8 trn2 NeuronCores.

Strategy: 2-way token x 4-way vocab sharding (grid (i, j), core c = i*4 + j).
  - Tokens (N=8192) split in 2 halves of T=4096; vocab (V=32000) split in 4
    quarters of VC=8000. Each core computes the partial exp-sum of its token
    half over its vocab quarter; the host adds the 4 partials per token and
    takes log (the cross-device logsumexp of the sharding hint, done on the
    scalar-sized partials host-side, standing in for the wrapper's
    all_reduce).
  - Per-core HBM traffic is ~46 MB (fp8 weights quarter + fp8 hidden half +
    f32 rowdot operands + broadcast bias), far below the tensor-engine time,
    so the kernel runs at the PE roofline: fp8e4 DoubleRow matmuls (K=256
    per pass, 0.5 cycles/column) = ~427 us of PE work per core.
  - Quantization to fp8 (weights pre-scaled by 64 for e4m3 range) happens on
    host; the device descales during the bias add, exactly matching the
    numerics of the on-device-converted fp8 baseline (~7.6e-5 rel err).
  - The target logit is computed exactly in f32: host gathers W[labels], the
    device does the per-token rowdot with an accumulating DVE
    scalar_tensor_tensor spread through group 1's slack. Bias added on host.

Device pipeline per (vocab-group g, token-block m, half): 16 DoubleRow
matmuls accumulate psum [128 tok x 2 banks x 500 vocab]; DVE does
(psum/64 + bias) in place; ACT computes exp with a fused row-sum
accumulator into s_cols. Per-bank-pair psum tiles (4 in flight) keep the
drain chain off the PE critical path.
"""
import numpy as np
import ml_dtypes
from contextlib import ExitStack

from concourse import bacc, mybir, tile
from concourse.bass_utils import run_bass_kernel_spmd

F32 = mybir.dt.float32
FP8 = mybir.dt.float8e4
Alu = mybir.AluOpType
Act = mybir.ActivationFunctionType

N_CORES = 8
N_TOK = 8192
D = 2048
V = 32000
P = 128

TOK_SPLIT = 2
VOC_SPLIT = 4
T = N_TOK // TOK_SPLIT        # 4096 tokens per core
VC = V // VOC_SPLIT           # 8000 vocab per core
KP = D // 256                 # 8 DoubleRow contraction passes of K=256
GV = 2000                     # vocab columns per W group (4 psum banks)
NG = VC // GV                 # 4 groups per core
MB = T // P                   # 32 token blocks per core
MBQ = MB // VOC_SPLIT         # 8 rowdot token blocks per core (1024 tokens)
BANK = 500                    # columns per psum bank

W_SCALE = 64.0                # fp8 weight pre-scale (e4m3 range)
FP8NP = ml_dtypes.float8_e4m3


def _build():
    nc = bacc.Bacc("TRN2", target_bir_lowering=False, debug=False)
    h_d = nc.declare_dram_parameter("h", [P, KP, 2, T], FP8, isOutput=False)
    h0a_d = nc.declare_dram_parameter("h0a", [P, KP * 2 * 256], FP8, isOutput=False)
    h0b_d = nc.declare_dram_parameter("h0b", [P, KP * 2 * 256], FP8, isOutput=False)
    h0_d = nc.declare_dram_parameter("h0", [P, KP * 2 * 256], FP8, isOutput=False)
    W_d = nc.declare_dram_parameter("W", [P, KP, 2, VC], FP8, isOutput=False)
    bias_d = nc.declare_dram_parameter("bias", [VC], F32, isOutput=False)
    brow_d = nc.declare_dram_parameter("brow", [GV // 2], mybir.dt.bfloat16,
                                       isOutput=False)
    hn_d = nc.declare_dram_parameter("hn", [MBQ * P, D], F32, isOutput=False)
    wg_d = nc.declare_dram_parameter("wg", [MBQ * P, D], F32, isOutput=False)
    s_out = nc.declare_dram_parameter("s_out", [P, MB], F32, isOutput=True)
    s_out2 = nc.declare_dram_parameter("s_out2", [P, NG * 2 + 1], F32,
                                       isOutput=True)
    t_out = nc.declare_dram_parameter("t_out", [P, MBQ], F32, isOutput=True)

    # h streams in token-chunks: each chunk carries ALL contraction passes
    # for 4 m-blocks, so the pipeline reaches full rate after ~1 MB of h
    HC = 512                                           # tokens per h chunk
    h_r = h_d[:].rearrange("p kp j (c t) -> c p kp j t", t=HC)
    W_r = W_d[:]                                       # [128, KP, 2, VC]

    with tile.TileContext(nc) as tc, ExitStack() as ctx:
        hpool = ctx.enter_context(tc.tile_pool(name="hT", bufs=1))
        wpool = ctx.enter_context(tc.tile_pool(name="w", bufs=2))
        bpool = ctx.enter_context(tc.tile_pool(name="bias", bufs=2))
        pspool = ctx.enter_context(tc.tile_pool(name="ps", bufs=4, space="PSUM"))
        epool = ctx.enter_context(tc.tile_pool(name="ejunk", bufs=2))
        hgpool = ctx.enter_context(tc.tile_pool(name="hg", bufs=2))
        wgpool = ctx.enter_context(tc.tile_pool(name="wgt", bufs=2))
        djpool = ctx.enter_context(tc.tile_pool(name="dj", bufs=1))
        acc = ctx.enter_context(tc.tile_pool(name="acc", bufs=1))

        s_cols = acc.tile([P, MB * NG * 2 + 1], F32, tag="scols")
        s_fin = acc.tile([P, MB], F32, tag="sfin")
        t_fin = acc.tile([P, MBQ], F32, tag="tfin")

        # Warm-up: dummy f32 matmuls on a zeroed tile keep PE busy (and ramp
        # its p-state to max) while the first real operands stream in. They
        # finish right as the first W group lands, so real matmuls start at
        # full clock with no idle gap.
        warm_state = {}

        def emit_warm(n_big, n_small):
            # dummy matmuls on a zeroed tile keep PE busy and its p-state
            # ramped while real operands stream in; also injected into the
            # first tiles' stutter windows
            if "ws" not in warm_state:
                ws = acc.tile([P, 512], F32, tag="warm")
                nc.vector.memset(ws[:], 0.0)
                wpt = pspool.tile([P, 2, 512], F32, tag="ps", name="wpt")
                warm_state["ws"] = ws
                warm_state["pt"] = wpt
            ws, wpt = warm_state["ws"], warm_state["pt"]
            for i in range(n_big):
                nc.tensor.matmul(wpt[:, i % 2, 0:512], ws[:, 0:128],
                                 ws[:, 0:512], start=True, stop=True)
            for i in range(n_small):
                nc.tensor.matmul(wpt[:, i % 2, 0:128], ws[:, 0:128],
                                 ws[:, 0:128], start=True, stop=True)

        # bias cols 0:1000 as one 4KB row -> on-chip broadcast via a K=1
        # ones-vector matmul into psum (PE is idle during startup anyway);
        # this drops the 0.5MB broadcast DMA from the gating transfer chain

        # preload the Exp activation table off the critical path: the first
        # real activation would otherwise pay a ~1.3us LoadActFuncSet inside
        # the first psum drain chain
        az = acc.tile([P, 1], F32, tag="actwarm")
        nc.vector.memset(az[:], 0.0)
        aj = acc.tile([P, 1], F32, tag="actjunk")
        nc.scalar.activation(aj[:], az[:], Act.Exp)

        emit_warm(2, 9)

        # bias cols 0:1000 as one 2KB bf16 row -> on-chip broadcast via a
        # K=1 ones-vector matmul (PE is idle during startup anyway); this
        # drops the 0.5MB broadcast DMA from the gating transfer chain
        ones = acc.tile([1, P], mybir.dt.bfloat16, tag="ones")
        nc.vector.memset(ones[0:1, :], 1.0)
        brow = acc.tile([1, GV // 2], mybir.dt.bfloat16, tag="brow")
        nc.sync.dma_start(brow[0:1, :], brow_d[:].partition_broadcast(1))
        bpt = pspool.tile([P, 2, 512], F32, tag="ps", name="bpt")
        for bk in range(2):
            nc.tensor.matmul(bpt[:, bk, 0:BANK], ones[0:1, :],
                             brow[0:1, bk * BANK:(bk + 1) * BANK],
                             start=True, stop=True)

        # startup order tuned for earliest full-rate PE: first token chunk,
        # then W group 0 in 512-col slivers (512 keeps full DMA bandwidth),
        # bias afterwards (the psum ring gives the first drains slack)
        # group 0 runs all half-0 tiles first (cols 0:1000), so only the
        # first two W slivers + first bias half gate the pipeline; the rest
        # streams in far ahead of its consumption
        hT = hpool.tile([P, KP, 2, T], FP8, tag="hT")
        h0T = acc.tile([P, KP, 2, 256], FP8, tag="h0T")
        wv0 = wpool.tile([P, KP, 2, GV], FP8, tag="w")
        bb0 = bpool.tile([P, GV], F32, tag="bias")
        nc.sync.dma_start(wv0[:, :, :, 0:512], W_r[:, :, :, 0:512])
        nc.sync.dma_start(
            h0T[:], h0_d[:].rearrange("p (kp j t) -> p kp j t", kp=KP, j=2))
        nc.sync.dma_start(wv0[:, :, :, 512:1024], W_r[:, :, :, 512:1024])
        nc.sync.dma_start(hT[:, :, :, 0:HC], h_r[0])
        nc.sync.dma_start(bb0[:, 0:GV // 2], bias_d[0:GV // 2].partition_broadcast(P))
        for c in range(1, T // HC):
            nc.sync.dma_start(hT[:, :, :, c * HC:(c + 1) * HC], h_r[c])
        # cols 1000:2000 and the second bias half are only consumed by the
        # half-1 sweep of group 0 (~60us in) -- load them after the h chunks
        nc.sync.dma_start(wv0[:, :, :, 1024:GV], W_r[:, :, :, 1024:GV])
        nc.sync.dma_start(bb0[:, GV // 2:GV],
                          bias_d[GV // 2:GV].partition_broadcast(P))

        # prefetch group 1 and the rowdot operands behind it
        wv1 = wpool.tile([P, KP, 2, GV], FP8, tag="w")
        nc.sync.dma_start(wv1[:], W_r[:, :, :, GV:2 * GV])
        bb1 = bpool.tile([P, GV], F32, tag="bias")
        nc.sync.dma_start(bb1[:], bias_d[GV:2 * GV].partition_broadcast(P))
        rowdot_io = []
        for mb in range(MBQ):
            hg = hgpool.tile([P, D], F32, tag="hg")
            nc.sync.dma_start(hg[:], hn_d[mb * P:(mb + 1) * P, :])
            wgt = wgpool.tile([P, D], F32, tag="wgt")
            nc.sync.dma_start(wgt[:], wg_d[mb * P:(mb + 1) * P, :])
            rowdot_io.append((hg, wgt))

        wtiles = [wv0, wv1]
        btiles = [bb0, bb1]
        for g in range(NG):
            wv, bb = wtiles[g], btiles[g]
            if g + 2 < NG:          # keep the double-buffer one group ahead
                wnx = wpool.tile([P, KP, 2, GV], FP8, tag="w")
                nc.sync.dma_start(
                    wnx[:], W_r[:, :, :, (g + 2) * GV:(g + 3) * GV])
                bnx = bpool.tile([P, GV], F32, tag="bias")
                nc.sync.dma_start(
                    bnx[:], bias_d[(g + 2) * GV:(g + 3) * GV].partition_broadcast(P))
                wtiles.append(wnx)
                btiles.append(bnx)

            def emit_rowdot(mb):
                # exact-f32 target logits: t = sum_d hn * W[label], one fused
                # DVE op per token block, spread through group 1's m-loop to
                # fit the per-tile DVE slack
                hg, wgt = rowdot_io[mb]
                dj = djpool.tile([P, D], F32, tag="dj", name="dj")
                nc.vector.scalar_tensor_tensor(
                    dj[:], hg[:], 1.0, wgt[:],
                    op0=Alu.mult, op1=Alu.mult,
                    accum_out=t_fin[:, mb:mb + 1])

            def emit_tile(m, half, split=False):
                if g == 0 and half == 0 and m < 2:
                    # first two blocks read the small contiguous h0 landing
                    lhsT = h0T[:, :, :, m * P:(m + 1) * P]
                else:
                    lhsT = hT[:, :, :, m * P:(m + 1) * P]
                col = m * (NG * 2) + g * 2 + half
                bbv = bb[:, half * 2 * BANK:(half + 1) * 2 * BANK]
                bbv = bbv.rearrange("p (b c) -> p b c", c=BANK)
                for bk in range(2):
                    if not split and bk == 1:
                        break
                    pt = pspool.tile([P, 2, 512], F32, tag="ps", name="pt")
                    bks = [bk] if split else [0, 1]
                    for b in bks:
                        c0 = half * (2 * BANK) + b * BANK
                        for kp in range(KP):
                            nc.tensor.matmul(
                                pt[:, b, 0:BANK], lhsT[:, kp, :, :],
                                wv[:, kp, :, c0:c0 + BANK],
                                start=(kp == 0), stop=(kp == KP - 1),
                                perf_mode=mybir.MatmulPerfMode.DoubleRow,
                            )
                    if split:
                        # 1-bank tiles: bank 0 drains while bank 1 still
                        # accumulates, halving the end-of-program chain
                        psb = pt[:, bk:bk + 1, 0:BANK]
                        nc.vector.scalar_tensor_tensor(
                            psb, psb, 1.0 / W_SCALE, bbv[:, bk:bk + 1, :],
                            op0=Alu.mult, op1=Alu.add)
                        et = epool.tile([P, 2, BANK], F32, tag="ejunk",
                                        name="et")
                        cb = col + bk
                        nc.scalar.activation(
                            et[:, 0:1, :], psb, Act.Exp,
                            accum_out=s_cols[:, cb:cb + 1])
                    else:
                        psl = pt[:, 0:2, 0:BANK]
                        nc.vector.scalar_tensor_tensor(
                            psl, psl, 1.0 / W_SCALE, bbv,
                            op0=Alu.mult, op1=Alu.add)
                        et = epool.tile([P, 2, BANK], F32, tag="ejunk",
                                        name="et")
                        nc.scalar.activation(
                            et[:], psl, Act.Exp,
                            accum_out=s_cols[:, col:col + 1])

            if g == 0:
                # half-outer: the first 32 tiles touch only W cols 0:1000 and
                # the first bias half, minimizing what gates the pipeline
                for half in range(2):
                    for m in range(MB):
                        emit_tile(m, half)
            else:
                if g == 2:
                    # t_fin is complete after group 1's rowdots; get its
                    # writeback off the critical-path tail
                    nc.sync.dma_start(t_out[:], t_fin[:])
                for m in range(MB):
                    if g == 1 and m % 4 == 0:
                        emit_rowdot(m // 4)
                    for half in range(2):
                        last = (g == NG - 1 and m == MB - 1 and half == 1)
                        emit_tile(m, half, split=last)
                    if g == NG - 1 and m < MB - 1:
                        nc.vector.tensor_reduce(
                            s_fin[:, m:m + 1],
                            s_cols[:, m * (NG * 2):(m + 1) * (NG * 2)],
                            axis=mybir.AxisListType.X, op=Alu.add)
                    if g == NG - 1 and m == MB - 2:
                        # all but the last token block are final: write them
                        # back now so only the last block's raw partials
                        # remain on the end-of-program critical path
                        nc.sync.dma_start(s_out[:, 0:MB - 1], s_fin[:, 0:MB - 1])

        # last token block: ship the 9 raw partials (the split final tile
        # uses two columns); host reduces
        nc.scalar.dma_start(
            s_out2[:], s_cols[:, (MB - 1) * (NG * 2):MB * (NG * 2) + 1])

    nc.compile()
    return nc


_NC_CACHE = {}


def _get_program():
    if "nc" not in _NC_CACHE:
        _NC_CACHE["nc"] = _build()
    return _NC_CACHE["nc"]


def _to_sbuf_layout(a):
    """[D, X] f32/fp8 -> [128, KP, 2, X] matching d = kp*256 + j*128 + ki."""
    X = a.shape[1]
    return np.ascontiguousarray(
        a.reshape(KP, 2, P, X).transpose(2, 0, 1, 3))


def kernel(hidden_states, head_weight, head_bias, loss_weight, labels,
           chunk_size=None, **_unused):
    hidden = np.asarray(hidden_states, dtype=np.float32)
    W = np.asarray(head_weight, dtype=np.float32)
    bias = np.asarray(head_bias, dtype=np.float32)
    lw = np.asarray(loss_weight, dtype=np.float32)
    labels = np.asarray(labels).astype(np.int64)

    assert hidden.shape == (N_TOK, D) and W.shape == (V, D)

    nc = _get_program()

    hq = hidden.astype(FP8NP)                       # [N, D] fp8
    Wq = (W * W_SCALE).astype(FP8NP)                # [V, D] fp8, x64
    Wg = W[labels]                                  # gathered rows [N, D] f32

    in_maps = []
    for c in range(N_CORES):
        i, j = divmod(c, VOC_SPLIT)
        tok = slice(i * T, (i + 1) * T)
        voc = slice(j * VC, (j + 1) * VC)
        # rowdot tokens: quarter j of token half i
        rtok = slice(i * T + j * MBQ * P, i * T + (j + 1) * MBQ * P)
        hsb = _to_sbuf_layout(hq[tok].T)
        in_maps.append(dict(
            h=hsb,
            h0=np.ascontiguousarray(hsb[:, :, :, 0:256]).reshape(P, -1),
            W=_to_sbuf_layout(Wq[voc].T),
            bias=np.ascontiguousarray(bias[voc]),
            brow=np.ascontiguousarray(bias[voc][0:GV // 2]).astype(ml_dtypes.bfloat16),
            hn=np.ascontiguousarray(hidden[rtok]),
            wg=np.ascontiguousarray(Wg[rtok]),
        ))
    res = run_bass_kernel_spmd(nc, in_maps, list(range(N_CORES)))

    # unshard + host-side combine (the scalar all_reduce of the hint):
    # sum the 4 vocab-quarter exp-sums per token, then logsumexp
    s = np.zeros((TOK_SPLIT, T), np.float64)
    tgt = np.zeros(N_TOK, np.float64)
    for c in range(N_CORES):
        i, j = divmod(c, VOC_SPLIT)
        r = res.results[c]
        sc = r["s_out"].astype(np.float64)          # [P, MB], token = m*128+p
        sc[:, MB - 1] = r["s_out2"].astype(np.float64).sum(-1)
        s[i] += sc.T.reshape(-1)
        rtok = slice(i * T + j * MBQ * P, i * T + (j + 1) * MBQ * P)
        tgt[rtok] = r["t_out"].T.reshape(-1).astype(np.float64)
    lse = np.log(s.reshape(-1))
    tgt = tgt + bias[labels].astype(np.float64)     # rowdot excludes bias
    nll = lse - tgt
    w64 = lw.astype(np.float64)
    loss = (w64 * nll).sum() / max(w64.sum(), 1.0)
    return np.float32(loss)

